# revision 14
# baseline (speedup 1.0000x reference)
"""Trainium2 Bass kernel for nn_Attention_39934605918652.

res[b] = W0 @ x0[b] + sum_{n=1..N-1} W2 @ tanh(W1a @ x0[b] + W1b @ x[b,n])

Key algebraic optimization: W2 does not depend on n, so
    sum_n W2 @ tanh(...) = W2 @ (sum_n tanh(...))
which removes the second big matmul (only a [B,H]x[H,F] remains).

Sharding: data-parallel over batch B=128 across 8 cores (16 batches/core),
weights replicated. No collectives.

v2 (this file): the dominant [F=512]-contraction matmul runs in fp8 e4m3
DoubleRow mode (256 contraction rows per instruction, 2x bf16 throughput).
W1b is host-scaled by 32 so its N(0, 1/1024) entries use the e4m3 range;
the ACT tanh compensates with its free scale=1/32 immediate. Measured
engine budget that motivated the restructure (baseline, 88.2us):
  TensorMatrix 61.4us / Scalar(ACT) 51.0us / Vector(DVE) 39.1us / GpSimd 0.
With fp8 halving the PE, ACT's 128 per-batch bias+tanh calls (385ns each,
~180ns/call overhead) became the bottleneck, so consume is split between
two strategies (KB_NS1 of 32 tiles use S1, rest S4):
  S1: 4 per-batch ACT calls, bias fused via the ACT bias port.
  S4: bias pre-added into PSUM by a K=16 one-hot matmul on the PE
      (h0T x bmask, bmask carries the 32x scale and zeroes the pad
      column), then ONE 1024-col plain tanh call (1.03us vs 1.59us).
The segmented free-dim reduce is split between DVE and the otherwise-idle
GpSimd engine (KB_NGP of 32 tiles; GpSimd runs elementwise at ~0.42
efficiency but in parallel).

Device layout (per core), f-major so the contraction dim sits on SBUF
partitions:
  xiT   [F=512, BL*256]  columns grouped 256 per batch (255 real + 1 zero
                         pad), fp8; loaded as DoubleRow pair tiles
                         [128, 2, cols] (partition p, pair i <-> f-row
                         fpair*256 + i*128 + p)
  x0T   [128, 4*BL]      host-packed f-chunks side by side, fp16
  w1bT  [F=512, H=1024]  (= 32 * W1[:, F:].T), fp8, pair tiles like xi
  w1aT  [F=512, H=1024]  (= W1[:, :F].T), fp16
  w2T   [H=1024, F=512]  (= W2.T), fp16
  w0T   [F=512, F=512]   (= W0.T), fp16
  bmask [16, 4*1024]     one-hot bias mask per quad: bmask[r, q*1024 +
                         b*256 + n] = 32 iff r == q*4+b and n != 255
Output res [BL=16, F=512] per core (batch-major); host concatenates.
"""

import os
import numpy as np
from contextlib import ExitStack

import concourse.bass as bass
import concourse.tile as tile
from concourse import bacc, mybir
from concourse.bass_utils import run_bass_kernel_spmd

N_CORES = 8
B, N, F, H = 128, 256, 512, 1024
BL = B // N_CORES          # 16 batches per core
NI = N - 1                 # 255 real columns per batch
NP = 256                   # padded columns per batch
NF = F // 128              # 4 f-chunks
FP = 2                     # 2 f-pair chunks (256 rows each, DoubleRow)
NH = H // 128              # 8 h-tiles
QUADS = BL // 4            # 4 batch-quads; per quad psum tile [128, 4*256]
WSCALE = 32.0              # host-side W1b/bias scale (ACT scale=1/32)

F32 = mybir.dt.float32
BF16 = mybir.dt.bfloat16
F16 = mybir.dt.float16
F8 = mybir.dt.float8e4
DR = mybir.MatmulPerfMode.DoubleRow

# Knobs (sweepable on hw):
#  KB_NS1A: tiles using S1a = 4 per-batch fused-bias ACT calls with the
#          ACT accumulator producing S directly (no DVE reduce; +187ns
#          engine time per call for the accumulator read).
#  KB_NS1: tiles using S1 = 4 per-batch fused-bias ACT calls + reduce.
#          Remaining tiles use S4 = PE one-hot bias matmul + one big
#          1024-col plain tanh + reduce.
#  Reduce variant for the S1/S4 tiles, spread over them:
#  KB_NTTR: reduces via DVE tensor_tensor_reduce 4x[128,256] (2-byte
#          operands may hit the DVE 2x mode).
#  KB_NGH: reduces prefaced by a GpSimd halving add (SBUF only), then a
#          half-width DVE reduce.  Rest: plain DVE reduce_sum.
#  KB_WARM: dummy [128,256]-col f32 matmuls during the DMA lead-in to
#          keep the PE activity monitor (HAM) at full clock.
#  KB_PPB: main PSUM pool bufs ([128,1024] f32 slots, 2 banks each).
# NOTE: KB_NTTR (tensor_tensor_reduce) measured on hw: the instruction
# wedges the device (NRT unrecoverable) — leave at 0.
NS1A = int(os.environ.get("KB_NS1A", "0"))
NS1 = int(os.environ.get("KB_NS1", "12"))
NTTR = int(os.environ.get("KB_NTTR", "0"))
NGH = int(os.environ.get("KB_NGH", "22"))
WARM_N = int(os.environ.get("KB_WARM", "4"))
PPB = int(os.environ.get("KB_PPB", "3"))


def _spread(idx, count, total=32):
    """True for `count` of `total` indices, evenly spread."""
    return (idx * count) // total != ((idx + 1) * count) // total


def _build_kernel():
    nc = bacc.Bacc(
        "TRN2", target_bir_lowering=False, debug=False, num_devices=N_CORES
    )

    xiT = nc.dram_tensor("xiT", [F, BL * NP], F8, kind="ExternalInput").ap()
    x0T = nc.dram_tensor("x0T", [128, NF * BL], F16, kind="ExternalInput").ap()
    w1bT = nc.dram_tensor("w1bT", [F, H], F8, kind="ExternalInput").ap()
    w1aT = nc.dram_tensor("w1aT", [F, H], F16, kind="ExternalInput").ap()
    w2T = nc.dram_tensor("w2T", [H, F], F16, kind="ExternalInput").ap()
    w0T = nc.dram_tensor("w0T", [F, F], F16, kind="ExternalInput").ap()
    bmaskT = nc.dram_tensor(
        "bmaskT", [BL, QUADS * 1024], F16, kind="ExternalInput"
    ).ap()
    res = nc.dram_tensor("res", [BL, F], F32, kind="ExternalOutput").ap()

    with tile.TileContext(nc) as tc:
        with ExitStack() as ctx:
            _kernel_body(ctx, tc, xiT, x0T, w1bT, w1aT, w2T, w0T, bmaskT, res)

    nc.compile()
    return nc


def _kernel_body(ctx, tc, xiT, x0T, w1bT, w1aT, w2T, w0T, bmaskT, res):
    nc = tc.nc
    Tanh = mybir.ActivationFunctionType.Tanh

    wpool = ctx.enter_context(tc.tile_pool(name="weights", bufs=1))

    def load(name, dram, rows, width, dt):
        tiles = []
        for c in range(rows // 128):
            t = wpool.tile([128, width], dt, tag=f"{name}_{c}", name=f"{name}_{c}")
            nc.sync.dma_start(t[:], dram[c * 128 : (c + 1) * 128, :])
            tiles.append(t)
        return tiles

    def load_pair(name, dram, fpair, cols):
        """DoubleRow pair tile [128, 2, len(cols)]: partition p, pair i
        holds dram row fpair*256 + i*128 + p."""
        t = wpool.tile([128, 2, cols.stop - cols.start], F8, tag=name, name=name)
        for i in range(2):
            r0 = fpair * 256 + i * 128
            nc.sync.dma_start(t[:, i, :], dram[r0 : r0 + 128, cols])
        return t

    # ---- DMA issue order = first-need order ----
    # Phase-2 wave 0 needs x0 (nothing), w1b, xi-c0-half; w1a (phase 1)
    # is deferred past them, then bmask/w0 (early epilogue), xi c1, w2.
    x0_all = wpool.tile([128, NF * BL], F16, tag="x0", name="x0_all")
    nc.sync.dma_start(x0_all[:], x0T[:])
    x0_sb = [x0_all[:, f * BL : (f + 1) * BL] for f in range(NF)]
    w1b_sb = [load_pair(f"w1b_{fp}", w1bT, fp, slice(0, H)) for fp in range(FP)]
    HC = BL * NP // 2
    xi_sb = [[None, None] for _ in range(FP)]
    for fp in range(FP):  # c0 halves: gate the first wave-0 tiles
        xi_sb[fp][0] = load_pair(f"xi_{fp}_0", xiT, fp, slice(0, HC))
    w1a_sb = load("w1a", w1aT, F, H, F16)
    bmask_sb = wpool.tile([BL, QUADS * 1024], F16, tag="bmask", name="bmask")
    nc.sync.dma_start(bmask_sb[:], bmaskT[:])
    w0_sb = load("w0", w0T, F, F, F16)
    for fp in range(FP):  # c1 halves: needed ~18us in, stream mid-wave-0
        xi_sb[fp][1] = load_pair(f"xi_{fp}_1", xiT, fp, slice(HC, 2 * HC))
    w2_sb = load("w2", w2T, H, F, F16)

    h0_sb = [
        wpool.tile([128, BL], F32, tag=f"h0_{h}", name=f"h0_{h}")
        for h in range(NH)
    ]
    h0T_sb = wpool.tile([BL, H], F16, tag="h0T", name="h0T")
    S_sb = [
        wpool.tile([128, BL], F16, tag=f"S_{h}", name=f"S_{h}")
        for h in range(NH)
    ]

    # One PSUM pool; every tile shares the tag so slots recycle.
    # Slot = [128, 4*NP] f32 = 2 banks. PPB main slots + 1 held by the
    # early-opened epilogue accumulator po.
    ppool = ctx.enter_context(tc.tile_pool(name="ps", bufs=PPB + 1, space="PSUM"))
    itpool = ctx.enter_context(tc.tile_pool(name="it", bufs=6))

    # ---- Phase 0: PE warm-up during the DMA lead-in ----
    # Dummy matmuls on zeros (no DMA dependency) keep the HAM activity
    # monitor warm so real matmuls issue at 2.4GHz from the start.
    if WARM_N:
        wz = wpool.tile([128, 256], F32, tag="warmz", name="warmz")
        nc.vector.memset(wz[:], 0.0)
        pw = ppool.tile([128, 256], F32, tag="ps", name="pwarm")
        for _ in range(WARM_N):
            nc.tensor.matmul(pw[:], wz[:, :128], wz[:], start=True, stop=True)

    # ---- Phase 0b: preload the tanh ACT table during the DMA lead-in
    tiny = wpool.tile([128, 1], F32, tag="tiny", name="tiny")
    nc.vector.memset(tiny[:], 0.0)
    nc.scalar.activation(tiny[:], tiny[:], Tanh)

    # ---- Phase 1 (issued later, between early wave-0 tiles; see below):
    def phase1a():
        # h0[h*128+p, b] = sum_f W1a[h, f] x0[b, f]; [128, BL] per h-tile.
        for h in range(NH):
            ph = ppool.tile([128, BL], F32, tag="ps", name=f"ph0_{h}")
            for f in range(NF):
                nc.tensor.matmul(
                    ph[:],
                    w1a_sb[f][:, h * 128 : (h + 1) * 128],
                    x0_sb[f],
                    start=(f == 0),
                    stop=(f == NF - 1),
                )
            nc.vector.tensor_copy(h0_sb[h][:], ph[:])

    def phase1b():
        # h0T[b, h] flipped variant for the S4 bias matmuls' stationary.
        ph = ppool.tile([BL, H], F32, tag="ps", name="ph0T")
        for hb in range(2):
            for f in range(NF):
                nc.tensor.matmul(
                    ph[:, hb * 512 : (hb + 1) * 512],
                    x0_sb[f],
                    w1a_sb[f][:, hb * 512 : (hb + 1) * 512],
                    start=(f == 0),
                    stop=(f == NF - 1),
                )
        with nc.allow_low_precision(reason="h0T feeds fp16 bias matmul"):
            nc.vector.tensor_copy(h0T_sb[:], ph[:])

    # ---- Phase 3 (early part): epilogue accumulator po, W0 term.
    # Opened mid-stream (needs only x0/w0); S-term matmuls are appended
    # as each h-tile's S completes; po's pool slot stays held throughout.
    po_box = [None]

    def epilogue_w0():
        po = ppool.tile([BL, F], F32, tag="ps", name="po")
        po_box[0] = po
        for f in range(NF):
            nc.tensor.matmul(
                po[:], x0_sb[f], w0_sb[f][:], start=(f == 0), stop=False
            )

    def epilogue_s(h, last):
        nc.tensor.matmul(
            po_box[0][:], S_sb[h][:], w2_sb[h][:], start=False, stop=last,
        )

    # Scratch for the tensor_tensor_reduce variant (all-2-byte operands,
    # packed, to stay eligible for the DVE 2x perf mode).
    zer_sb = wpool.tile([128, NP], BF16, tag="zer", name="zer")
    nc.vector.memset(zer_sb[:], 0.0)
    junk_sb = wpool.tile([128, NP], BF16, tag="junk", name="junk")

    # ---- Phase 2: hi matmul (fp8 DoubleRow) + bias + tanh + reduce ----
    def consume(h, q, pb, cls, red):
        it = itpool.tile([128, 4 * NP], BF16, tag="it", name=f"it_{h}_{q}")
        s1ish = cls in ("s1", "s1a")
        nb = NI if s1ish else NP
        with nc.allow_low_precision(
            reason="S accumulated in 16-bit to feed the 16-bit output matmul"
        ):
            if s1ish:
                # per-batch ACT, bias via the ACT bias port, skip pad col;
                # s1a also reads the ACT accumulator = the row sum = S col.
                for bl in range(4):
                    b = q * 4 + bl
                    acc = (
                        S_sb[h][:, b : b + 1] if cls == "s1a" else None
                    )
                    nc.scalar.activation(
                        it[:, bl * NP : bl * NP + NI],
                        pb[:, bl * NP : bl * NP + NI],
                        Tanh,
                        bias=h0_sb[h][:, b : b + 1],
                        scale=1.0 / WSCALE,
                        accum_out=acc,
                    )
                if cls == "s1a":
                    return
            else:
                # S4: bias already in PSUM (one-hot matmul, pad col exact 0
                # since bmask zeroes it and tanh(0)=0): one big tanh call.
                nc.scalar.activation(it[:], pb[:], Tanh, scale=1.0 / WSCALE)
            scol = S_sb[h][:, q * 4 : (q + 1) * 4]
            view = it[:].rearrange("p (b n) -> p b n", b=4)
            if red == "ttr":
                # 4 contiguous per-batch fused add+reduce on DVE.
                for bl in range(4):
                    nc.vector.tensor_tensor_reduce(
                        junk_sb[:, :nb],
                        it[:, bl * NP : bl * NP + nb],
                        zer_sb[:, :nb],
                        1.0,
                        0.0,
                        mybir.AluOpType.add,
                        mybir.AluOpType.add,
                        accum_out=S_sb[h][:, q * 4 + bl : q * 4 + bl + 1],
                    )
            elif red == "gph":
                # GpSimd halving add (SBUF-only), then half-width DVE reduce.
                hb = nb // 2
                nc.gpsimd.tensor_add(
                    view[:, :, :hb],
                    view[:, :, :hb],
                    view[:, :, nb - hb : nb],
                )
                nc.vector.reduce_sum(
                    scol, view[:, :, : nb - hb], axis=mybir.AxisListType.X
                )
            else:
                nc.vector.reduce_sum(
                    scol, view[:, :, :nb], axis=mybir.AxisListType.X
                )

    def mm_tile(pb, h, q, wave, s1ish):
        # 2 DoubleRow matmuls per 512-col block: fpair 0 starts, fpair 1
        # accumulates; S4 appends the K=16 one-hot bias matmul.
        for bk in range(2):
            out = pb[:, bk * 512 : (bk + 1) * 512]
            for fp in range(FP):
                cols = slice(
                    (q % 2) * 4 * NP + bk * 512,
                    (q % 2) * 4 * NP + (bk + 1) * 512,
                )
                nc.tensor.matmul(
                    out,
                    w1b_sb[fp][:, :, h * 128 : (h + 1) * 128],
                    xi_sb[fp][wave][:, :, cols],
                    start=(fp == 0),
                    stop=(fp == FP - 1) and s1ish,
                    perf_mode=DR,
                )
            if not s1ish:
                nc.tensor.matmul(
                    out,
                    h0T_sb[:, h * 128 : (h + 1) * 128],
                    bmask_sb[:, q * 1024 + bk * 512 : q * 1024 + (bk + 1) * 512],
                    start=False,
                    stop=True,
                )

    # PE program order: a few wave-0 tiles first (gated only on w1b +
    # xi-c0, ~4.4us), then phase 1 (w1a lands ~7.4us), then the rest.
    # Epilogue W0 slots in once w0 has landed (~11us); S-term matmuls
    # trail each h's last consume by 2 h-tiles.
    sched = []
    for wave in range(QUADS // 2):
        for h in range(NH):
            for q in (2 * wave, 2 * wave + 1):
                sched.append((wave, h, q))

    # First LEAD tiles are matmul-only (forced S1a: no h0T/PE dependency),
    # then phase 1 runs on the PE while their consumes drain; this keeps
    # the PE busy from ~4.4us (w1b+xi-c0 landed) while w1a streams.
    LEAD = 3  # = PPB: tiles 0..LEAD-1 hold the main PSUM slots; phase 1
    # transiently borrows the last slot.

    # Deficit-spread class labels (s1a/s1/s4) and reduce variants
    # (ttr/gph/plain) across the 32 tiles. The first LEAD tiles must be
    # non-S4 (their matmuls precede phase 1 / h0T in the PE stream).
    counts = {"s1a": NS1A, "s1": NS1}
    if counts["s1a"] + counts["s1"] < LEAD:
        counts["s1"] = LEAD - counts["s1a"]
    counts["s4"] = 32 - counts["s1a"] - counts["s1"]
    labels = []
    used = {k: 0 for k in counts}
    for pos in range(32):
        opts = [k for k in counts if used[k] < counts[k]]
        if pos < LEAD:
            opts = [k for k in opts if k != "s4"] or ["s1a"]
        pick = max(opts, key=lambda k: counts[k] * (pos + 1) / 32 - used[k])
        used[pick] = used.get(pick, 0) + 1
        labels.append(pick)
    nred = sum(1 for l in labels if l != "s1a")
    rcounts = {"ttr": min(NTTR, nred), "gph": min(NGH, max(nred - NTTR, 0))}
    rcounts["plain"] = nred - rcounts["ttr"] - rcounts["gph"]
    rlabels = []
    rused = {k: 0 for k in rcounts}
    for pos in range(nred):
        opts = [k for k in rcounts if rused[k] < rcounts[k]]
        pick = max(opts, key=lambda k: rcounts[k] * (pos + 1) / nred - rused[k])
        rused[pick] += 1
        rlabels.append(pick)
    riter = iter(rlabels)
    plan = [(l, next(riter) if l != "s1a" else None) for l in labels]

    done_h = []
    deferred = []
    for pos, (wave, h, q) in enumerate(sched):
        if pos == LEAD:
            phase1a()
            phase1b()
            for args in deferred:
                consume(*args)
        cls, red = plan[pos]
        pb = ppool.tile([128, 4 * NP], F32, tag="ps", name=f"pb_{h}_{q}")
        mm_tile(pb, h, q, wave, cls in ("s1", "s1a"))
        if pos < LEAD:
            deferred.append((h, q, pb, cls, red))
            continue
        consume(h, q, pb, cls, red)
        if pos == 11:
            epilogue_w0()
        # wave-1 h-tiles complete in order; issue their S matmul 2 behind.
        if wave == 1 and q == 2 * wave + 1 and h >= 2:
            epilogue_s(h - 2, last=False)
            done_h.append(h - 2)

    for h in range(NH):
        if h not in done_h:
            epilogue_s(h, last=(h == NH - 1))

    rt = itpool.tile([BL, F], F32, tag="rt", name="rt")
    nc.vector.tensor_copy(rt[:], po_box[0][:])
    nc.sync.dma_start(res[:], rt[:])


_NC_CACHE = {}


def _get_nc():
    key = ("v3", NS1A, NS1, NTTR, NGH, WARM_N, PPB)
    if key not in _NC_CACHE:
        _NC_CACHE[key] = _build_kernel()
    return _NC_CACHE[key]


def _make_in_maps(x, W1, W2, W0):
    import ml_dtypes

    f8 = ml_dtypes.float8_e4m3
    x = np.ascontiguousarray(np.asarray(x, dtype=np.float32))
    W1 = np.asarray(W1, dtype=np.float32)
    W2 = np.asarray(W2, dtype=np.float32)
    W0 = np.asarray(W0, dtype=np.float32)

    w1aT = np.ascontiguousarray(W1[:, :F].T).astype(np.float16)        # [F, H]
    w1bT = np.ascontiguousarray(W1[:, F:].T * WSCALE).astype(f8)       # [F, H]
    w2T = np.ascontiguousarray(W2.T).astype(np.float16)                # [H, F]
    w0T = np.ascontiguousarray(W0.T).astype(np.float16)                # [F, F]

    # bmask[r, q*1024 + b*256 + n] = WSCALE iff r == q*4+b and n != 255
    bmask = np.zeros((BL, QUADS, 4, NP), dtype=np.float16)
    for qq in range(QUADS):
        for bb in range(4):
            bmask[qq * 4 + bb, qq, bb, :NI] = WSCALE
    bmask = bmask.reshape(BL, QUADS * 1024)

    in_maps = []
    for i in range(N_CORES):
        xc = x[i * BL : (i + 1) * BL]               # [BL, N, F]
        # packed [128, NF*BL]: row p, block f holds x0T[f*128+p, :]
        x0Tc = np.ascontiguousarray(
            xc[:, 0, :].T.reshape(NF, 128, BL).transpose(1, 0, 2).reshape(128, NF * BL)
        ).astype(np.float16)
        pad = np.zeros((BL, NP, F), dtype=np.float32)
        pad[:, :NI, :] = xc[:, 1:, :]
        xiTc = np.ascontiguousarray(pad.reshape(BL * NP, F).T).astype(f8)
        in_maps.append(
            {
                "xiT": xiTc,
                "x0T": x0Tc,
                "w1bT": w1bT,
                "w1aT": w1aT,
                "w2T": w2T,
                "w0T": w0T,
                "bmaskT": bmask,
            }
        )
    return in_maps


def _gather(results):
    out = np.empty((B, F), dtype=np.float32)
    for i in range(N_CORES):
        out[i * BL : (i + 1) * BL] = results[i]["res"]
    return out


def kernel(x, W1, W2, W0):
    nc = _get_nc()
    in_maps = _make_in_maps(x, W1, W2, W0)
    res = run_bass_kernel_spmd(nc, in_maps, list(range(N_CORES)))
    return _gather(res.results)


def kernel_profiled(x, W1, W2, W0, **trace_kwargs):
    """Like kernel() but with NTFF profiling; returns (out, exec_time_ns)."""
    nc = _get_nc()
    in_maps = _make_in_maps(x, W1, W2, W0)
    res = run_bass_kernel_spmd(
        nc, in_maps, list(range(N_CORES)), trace=True, **trace_kwargs
    )
    return _gather(res.results), res.exec_time_ns


# revision 16
# speedup vs baseline: 1.0004x; 1.0004x over previous
"""Trainium2 Bass kernel for nn_Attention_39934605918652.

res[b] = W0 @ x0[b] + sum_{n=1..N-1} W2 @ tanh(W1a @ x0[b] + W1b @ x[b,n])

Key algebraic optimization: W2 does not depend on n, so
    sum_n W2 @ tanh(...) = W2 @ (sum_n tanh(...))
which removes the second big matmul (only a [B,H]x[H,F] remains).

Sharding: data-parallel over batch B=128 across 8 cores (16 batches/core),
weights replicated. No collectives.

v2 (this file): the dominant [F=512]-contraction matmul runs in fp8 e4m3
DoubleRow mode (256 contraction rows per instruction, 2x bf16 throughput).
W1b is host-scaled by 32 so its N(0, 1/1024) entries use the e4m3 range;
the ACT tanh compensates with its free scale=1/32 immediate. Measured
engine budget that motivated the restructure (baseline, 88.2us):
  TensorMatrix 61.4us / Scalar(ACT) 51.0us / Vector(DVE) 39.1us / GpSimd 0.
With fp8 halving the PE, ACT's 128 per-batch bias+tanh calls (385ns each,
~180ns/call overhead) became the bottleneck, so consume is split between
two strategies (KB_NS1 of 32 tiles use S1, rest S4):
  S1: 4 per-batch ACT calls, bias fused via the ACT bias port.
  S4: bias pre-added into PSUM by a K=16 one-hot matmul on the PE
      (h0T x bmask, bmask carries the 32x scale and zeroes the pad
      column), then ONE 1024-col plain tanh call (1.03us vs 1.59us).
The segmented free-dim reduce is split between DVE and the otherwise-idle
GpSimd engine (KB_NGP of 32 tiles; GpSimd runs elementwise at ~0.42
efficiency but in parallel).

Device layout (per core), f-major so the contraction dim sits on SBUF
partitions:
  xiT   [F=512, BL*256]  columns grouped 256 per batch (255 real + 1 zero
                         pad), fp8; loaded as DoubleRow pair tiles
                         [128, 2, cols] (partition p, pair i <-> f-row
                         fpair*256 + i*128 + p)
  x0T   [128, 4*BL]      host-packed f-chunks side by side, fp16
  w1bT  [F=512, H=1024]  (= 32 * W1[:, F:].T), fp8, pair tiles like xi
  w1aT  [F=512, H=1024]  (= W1[:, :F].T), fp16
  w2T   [H=1024, F=512]  (= W2.T), fp16
  w0T   [F=512, F=512]   (= W0.T), fp16
  bmask [16, 4*1024]     one-hot bias mask per quad: bmask[r, q*1024 +
                         b*256 + n] = 32 iff r == q*4+b and n != 255
Output res [BL=16, F=512] per core (batch-major); host concatenates.
"""

import os
import numpy as np
from contextlib import ExitStack

import concourse.bass as bass
import concourse.tile as tile
from concourse import bacc, mybir
from concourse.bass_utils import run_bass_kernel_spmd

N_CORES = 8
B, N, F, H = 128, 256, 512, 1024
BL = B // N_CORES          # 16 batches per core
NI = N - 1                 # 255 real columns per batch
NP = 256                   # padded columns per batch
NF = F // 128              # 4 f-chunks
FP = 2                     # 2 f-pair chunks (256 rows each, DoubleRow)
NH = H // 128              # 8 h-tiles
QUADS = BL // 4            # 4 batch-quads; per quad psum tile [128, 4*256]
WSCALE = 32.0              # host-side W1b/bias scale (ACT scale=1/32)

F32 = mybir.dt.float32
BF16 = mybir.dt.bfloat16
F16 = mybir.dt.float16
F8 = mybir.dt.float8e4
DR = mybir.MatmulPerfMode.DoubleRow

# Knobs (sweepable on hw):
#  KB_NS1A: tiles using S1a = 4 per-batch fused-bias ACT calls with the
#          ACT accumulator producing S directly (no DVE reduce; +187ns
#          engine time per call for the accumulator read).
#  KB_NS1: tiles using S1 = 4 per-batch fused-bias ACT calls + reduce.
#          Remaining tiles use S4 = PE one-hot bias matmul + one big
#          1024-col plain tanh + reduce.
#  Reduce variant for the S1/S4 tiles, spread over them:
#  KB_NTTR: reduces via DVE tensor_tensor_reduce 4x[128,256] (2-byte
#          operands may hit the DVE 2x mode).
#  KB_NGH: reduces prefaced by a GpSimd halving add (SBUF only), then a
#          half-width DVE reduce.  Rest: plain DVE reduce_sum.
#  KB_WARM: dummy [128,256]-col f32 matmuls during the DMA lead-in to
#          keep the PE activity monitor (HAM) at full clock.
#  KB_PPB: main PSUM pool bufs ([128,1024] f32 slots, 2 banks each).
# NOTE: KB_NTTR (tensor_tensor_reduce) measured on hw: the instruction
# wedges the device (NRT unrecoverable) — leave at 0.
NS1A = int(os.environ.get("KB_NS1A", "0"))
NS1 = int(os.environ.get("KB_NS1", "12"))
NTTR = int(os.environ.get("KB_NTTR", "0"))
NGH = int(os.environ.get("KB_NGH", "22"))
WARM_N = int(os.environ.get("KB_WARM", "8"))
PPB = int(os.environ.get("KB_PPB", "4"))


def _spread(idx, count, total=32):
    """True for `count` of `total` indices, evenly spread."""
    return (idx * count) // total != ((idx + 1) * count) // total


def _build_kernel():
    nc = bacc.Bacc(
        "TRN2", target_bir_lowering=False, debug=False, num_devices=N_CORES
    )

    xiT = nc.dram_tensor("xiT", [F, BL * NP], F8, kind="ExternalInput").ap()
    x0T = nc.dram_tensor("x0T", [128, NF * BL], F16, kind="ExternalInput").ap()
    w1bT = nc.dram_tensor("w1bT", [F, H], F8, kind="ExternalInput").ap()
    w1aT = nc.dram_tensor("w1aT", [F, H], F16, kind="ExternalInput").ap()
    w2T = nc.dram_tensor("w2T", [H, F], F16, kind="ExternalInput").ap()
    w0T = nc.dram_tensor("w0T", [F, F], F16, kind="ExternalInput").ap()
    bmaskT = nc.dram_tensor(
        "bmaskT", [BL, QUADS * 1024], F16, kind="ExternalInput"
    ).ap()
    res = nc.dram_tensor("res", [BL, F], F32, kind="ExternalOutput").ap()

    with tile.TileContext(nc) as tc:
        with ExitStack() as ctx:
            _kernel_body(ctx, tc, xiT, x0T, w1bT, w1aT, w2T, w0T, bmaskT, res)

    nc.compile()
    return nc


def _kernel_body(ctx, tc, xiT, x0T, w1bT, w1aT, w2T, w0T, bmaskT, res):
    nc = tc.nc
    Tanh = mybir.ActivationFunctionType.Tanh

    wpool = ctx.enter_context(tc.tile_pool(name="weights", bufs=1))

    def load(name, dram, rows, width, dt):
        tiles = []
        for c in range(rows // 128):
            t = wpool.tile([128, width], dt, tag=f"{name}_{c}", name=f"{name}_{c}")
            nc.sync.dma_start(t[:], dram[c * 128 : (c + 1) * 128, :])
            tiles.append(t)
        return tiles

    def load_pair(name, dram, fpair, cols):
        """DoubleRow pair tile [128, 2, len(cols)]: partition p, pair i
        holds dram row fpair*256 + i*128 + p."""
        t = wpool.tile([128, 2, cols.stop - cols.start], F8, tag=name, name=name)
        for i in range(2):
            r0 = fpair * 256 + i * 128
            nc.sync.dma_start(t[:, i, :], dram[r0 : r0 + 128, cols])
        return t

    # ---- DMA issue order = first-need order ----
    # Phase-2 wave 0 needs x0 (nothing), w1b, xi-c0-half; w1a (phase 1)
    # is deferred past them, then bmask/w0 (early epilogue), xi c1, w2.
    x0_all = wpool.tile([128, NF * BL], F16, tag="x0", name="x0_all")
    nc.sync.dma_start(x0_all[:], x0T[:])
    x0_sb = [x0_all[:, f * BL : (f + 1) * BL] for f in range(NF)]
    # xi per-quad tiles [128, 2, 1024]; interleave with w1b so the very
    # first tile's operands (w1b_0 + xi_0_q0 + w1b_1 + xi_1_q0, ~1MB)
    # land ~3us in.
    QW = 4 * NP  # 1024 columns per quad
    w1b_sb = [None, None]
    xi_sb = [[None] * QUADS for _ in range(FP)]
    w1b_sb[0] = load_pair("w1b_0", w1bT, 0, slice(0, H))
    xi_sb[0][0] = load_pair("xi_0_0", xiT, 0, slice(0, QW))
    w1b_sb[1] = load_pair("w1b_1", w1bT, 1, slice(0, H))
    xi_sb[1][0] = load_pair("xi_1_0", xiT, 1, slice(0, QW))
    for fp in range(FP):
        xi_sb[fp][1] = load_pair(f"xi_{fp}_1", xiT, fp, slice(QW, 2 * QW))
    w1a_sb = load("w1a", w1aT, F, H, F16)
    bmask_sb = wpool.tile([BL, QUADS * 1024], F16, tag="bmask", name="bmask")
    nc.sync.dma_start(bmask_sb[:], bmaskT[:])
    w0_sb = load("w0", w0T, F, F, F16)
    for q in (2, 3):  # wave-1 quads: needed ~18us in, stream mid-wave-0
        for fp in range(FP):
            xi_sb[fp][q] = load_pair(
                f"xi_{fp}_{q}", xiT, fp, slice(q * QW, (q + 1) * QW)
            )
    w2_sb = load("w2", w2T, H, F, F16)

    h0_sb = [
        wpool.tile([128, BL], F32, tag=f"h0_{h}", name=f"h0_{h}")
        for h in range(NH)
    ]
    h0T_sb = wpool.tile([BL, H], F16, tag="h0T", name="h0T")
    S_sb = [
        wpool.tile([128, BL], F16, tag=f"S_{h}", name=f"S_{h}")
        for h in range(NH)
    ]

    # One PSUM pool; every tile shares the tag so slots recycle.
    # Slot = [128, 4*NP] f32 = 2 banks; PPB slots = the full 8 banks.
    # Warm-up, phase 1, and the epilogue matmul groups borrow slots
    # transiently; the epilogue accumulates in SBUF via DVE.
    ppool = ctx.enter_context(tc.tile_pool(name="ps", bufs=PPB, space="PSUM"))
    itpool = ctx.enter_context(tc.tile_pool(name="it", bufs=6))

    # ---- Phase 0: PE warm-up during the DMA lead-in ----
    # Dummy matmuls on zeros (no DMA dependency) keep the HAM activity
    # monitor warm so real matmuls issue at 2.4GHz from the start.
    if WARM_N:
        wz = wpool.tile([128, 128], F32, tag="warmz", name="warmz")
        nc.vector.memset(wz[:], 0.0)
        pw = ppool.tile([128, 128], F32, tag="ps", name="pwarm")
        for _ in range(WARM_N):
            nc.tensor.matmul(pw[:], wz[:], wz[:], start=True, stop=True)

    # ---- Phase 0b: preload the tanh ACT table during the DMA lead-in
    tiny = wpool.tile([128, 1], F32, tag="tiny", name="tiny")
    nc.vector.memset(tiny[:], 0.0)
    nc.scalar.activation(tiny[:], tiny[:], Tanh)

    # ---- Phase 1 (issued later, between early wave-0 tiles; see below):
    def phase1a():
        # h0[h*128+p, b] = sum_f W1a[h, f] x0[b, f]; [128, BL] per h-tile.
        for h in range(NH):
            ph = ppool.tile([128, BL], F32, tag="ps", name=f"ph0_{h}")
            for f in range(NF):
                nc.tensor.matmul(
                    ph[:],
                    w1a_sb[f][:, h * 128 : (h + 1) * 128],
                    x0_sb[f],
                    start=(f == 0),
                    stop=(f == NF - 1),
                )
            nc.vector.tensor_copy(h0_sb[h][:], ph[:])

    def phase1b():
        # h0T[b, h] flipped variant for the S4 bias matmuls' stationary.
        ph = ppool.tile([BL, H], F32, tag="ps", name="ph0T")
        for hb in range(2):
            for f in range(NF):
                nc.tensor.matmul(
                    ph[:, hb * 512 : (hb + 1) * 512],
                    x0_sb[f],
                    w1a_sb[f][:, hb * 512 : (hb + 1) * 512],
                    start=(f == 0),
                    stop=(f == NF - 1),
                )
        with nc.allow_low_precision(reason="h0T feeds fp16 bias matmul"):
            nc.vector.tensor_copy(h0T_sb[:], ph[:])

    # ---- Phase 3: epilogue res = W0 x0 + W2 S, accumulated in SBUF.
    # Each 4-matmul group borrows a psum slot transiently and DVE folds
    # it into rt_acc, so no slot is held across phase 2.
    rt_acc = wpool.tile([BL, F], F32, tag="rt", name="rt_acc")

    def epilogue_w0():
        pw = ppool.tile([BL, F], F32, tag="ps", name="po_w0")
        for f in range(NF):
            nc.tensor.matmul(
                pw[:], x0_sb[f], w0_sb[f][:], start=(f == 0), stop=(f == NF - 1)
            )
        nc.vector.tensor_copy(rt_acc[:], pw[:])

    def epilogue_s_group(hs, name):
        pg = ppool.tile([BL, F], F32, tag="ps", name=name)
        for i, h in enumerate(hs):
            nc.tensor.matmul(
                pg[:], S_sb[h][:], w2_sb[h][:],
                start=(i == 0), stop=(i == len(hs) - 1),
            )
        nc.vector.tensor_add(rt_acc[:], rt_acc[:], pg[:])

    # Scratch for the tensor_tensor_reduce variant (all-2-byte operands,
    # packed, to stay eligible for the DVE 2x perf mode).
    zer_sb = wpool.tile([128, NP], BF16, tag="zer", name="zer")
    nc.vector.memset(zer_sb[:], 0.0)
    junk_sb = wpool.tile([128, NP], BF16, tag="junk", name="junk")

    # ---- Phase 2: hi matmul (fp8 DoubleRow) + bias + tanh + reduce ----
    def consume(h, q, pb, cls, red):
        it = itpool.tile([128, 4 * NP], BF16, tag="it", name=f"it_{h}_{q}")
        s1ish = cls in ("s1", "s1a")
        nb = NI if s1ish else NP
        with nc.allow_low_precision(
            reason="S accumulated in 16-bit to feed the 16-bit output matmul"
        ):
            if s1ish:
                # per-batch ACT, bias via the ACT bias port, skip pad col;
                # s1a also reads the ACT accumulator = the row sum = S col.
                for bl in range(4):
                    b = q * 4 + bl
                    acc = (
                        S_sb[h][:, b : b + 1] if cls == "s1a" else None
                    )
                    nc.scalar.activation(
                        it[:, bl * NP : bl * NP + NI],
                        pb[:, bl * NP : bl * NP + NI],
                        Tanh,
                        bias=h0_sb[h][:, b : b + 1],
                        scale=1.0 / WSCALE,
                        accum_out=acc,
                    )
                if cls == "s1a":
                    return
            else:
                # S4: bias already in PSUM (one-hot matmul, pad col exact 0
                # since bmask zeroes it and tanh(0)=0): one big tanh call.
                nc.scalar.activation(it[:], pb[:], Tanh, scale=1.0 / WSCALE)
            scol = S_sb[h][:, q * 4 : (q + 1) * 4]
            view = it[:].rearrange("p (b n) -> p b n", b=4)
            if red == "ttr":
                # 4 contiguous per-batch fused add+reduce on DVE.
                for bl in range(4):
                    nc.vector.tensor_tensor_reduce(
                        junk_sb[:, :nb],
                        it[:, bl * NP : bl * NP + nb],
                        zer_sb[:, :nb],
                        1.0,
                        0.0,
                        mybir.AluOpType.add,
                        mybir.AluOpType.add,
                        accum_out=S_sb[h][:, q * 4 + bl : q * 4 + bl + 1],
                    )
            elif red == "gph":
                # GpSimd halving add (SBUF-only), then half-width DVE reduce.
                hb = nb // 2
                nc.gpsimd.tensor_add(
                    view[:, :, :hb],
                    view[:, :, :hb],
                    view[:, :, nb - hb : nb],
                )
                nc.vector.reduce_sum(
                    scol, view[:, :, : nb - hb], axis=mybir.AxisListType.X
                )
            else:
                nc.vector.reduce_sum(
                    scol, view[:, :, :nb], axis=mybir.AxisListType.X
                )

    def mm_tile(pb, h, q, s1ish):
        # 2 DoubleRow matmuls per 512-col block: fpair 0 starts, fpair 1
        # accumulates; S4 appends the K=16 one-hot bias matmul.
        for bk in range(2):
            out = pb[:, bk * 512 : (bk + 1) * 512]
            for fp in range(FP):
                nc.tensor.matmul(
                    out,
                    w1b_sb[fp][:, :, h * 128 : (h + 1) * 128],
                    xi_sb[fp][q][:, :, bk * 512 : (bk + 1) * 512],
                    start=(fp == 0),
                    stop=(fp == FP - 1) and s1ish,
                    perf_mode=DR,
                )
            if not s1ish:
                nc.tensor.matmul(
                    out,
                    h0T_sb[:, h * 128 : (h + 1) * 128],
                    bmask_sb[:, q * 1024 + bk * 512 : q * 1024 + (bk + 1) * 512],
                    start=False,
                    stop=True,
                )

    # PE program order: a few wave-0 tiles first (gated only on w1b +
    # xi-q0, ~3us), then phase 1 (w1a lands ~6.5us), then the rest.
    # Wave 1 walks h DESCENDING so S[7..4] complete early and their
    # epilogue group can issue mid-stream; only the {3..0} group trails
    # the final consume.
    sched = []
    for wave in range(QUADS // 2):
        hs = range(NH) if wave == 0 else range(NH - 1, -1, -1)
        for h in hs:
            for q in (2 * wave, 2 * wave + 1):
                sched.append((wave, h, q))

    # First LEAD tiles are matmul-only (forced S1a: no h0T/PE dependency),
    # then phase 1 runs on the PE while their consumes drain; this keeps
    # the PE busy from ~4.4us (w1b+xi-c0 landed) while w1a streams.
    LEAD = 3  # = PPB: tiles 0..LEAD-1 hold the main PSUM slots; phase 1
    # transiently borrows the last slot.

    # Deficit-spread class labels (s1a/s1/s4) and reduce variants
    # (ttr/gph/plain) across the 32 tiles. The first LEAD tiles must be
    # non-S4 (their matmuls precede phase 1 / h0T in the PE stream).
    counts = {"s1a": NS1A, "s1": NS1}
    if counts["s1a"] + counts["s1"] < LEAD:
        counts["s1"] = LEAD - counts["s1a"]
    counts["s4"] = 32 - counts["s1a"] - counts["s1"]
    labels = []
    used = {k: 0 for k in counts}
    for pos in range(32):
        opts = [k for k in counts if used[k] < counts[k]]
        if pos < LEAD:
            opts = [k for k in opts if k != "s4"] or ["s1a"]
        pick = max(opts, key=lambda k: counts[k] * (pos + 1) / 32 - used[k])
        used[pick] = used.get(pick, 0) + 1
        labels.append(pick)
    nred = sum(1 for l in labels if l != "s1a")
    rcounts = {"ttr": min(NTTR, nred), "gph": min(NGH, max(nred - NTTR, 0))}
    rcounts["plain"] = nred - rcounts["ttr"] - rcounts["gph"]
    rlabels = []
    rused = {k: 0 for k in rcounts}
    for pos in range(nred):
        opts = [k for k in rcounts if rused[k] < rcounts[k]]
        pick = max(opts, key=lambda k: rcounts[k] * (pos + 1) / nred - rused[k])
        rused[pick] += 1
        rlabels.append(pick)
    riter = iter(rlabels)
    plan = [(l, next(riter) if l != "s1a" else None) for l in labels]

    deferred = []
    for pos, (wave, h, q) in enumerate(sched):
        if pos == LEAD:
            phase1a()
            phase1b()
            for args in deferred:
                consume(*args)
        cls, red = plan[pos]
        pb = ppool.tile([128, 4 * NP], F32, tag="ps", name=f"pb_{h}_{q}")
        mm_tile(pb, h, q, cls in ("s1", "s1a"))
        if pos < LEAD:
            deferred.append((h, q, pb, cls, red))
            continue
        consume(h, q, pb, cls, red)
        if pos == 11:
            epilogue_w0()
        if pos == 25:
            epilogue_s_group([7, 6, 5, 4], "po_sA")

    epilogue_s_group([3, 2, 1, 0], "po_sB")
    nc.sync.dma_start(res[:], rt_acc[:])


_NC_CACHE = {}


def _get_nc():
    key = ("v3", NS1A, NS1, NTTR, NGH, WARM_N, PPB)
    if key not in _NC_CACHE:
        _NC_CACHE[key] = _build_kernel()
    return _NC_CACHE[key]


def _make_in_maps(x, W1, W2, W0):
    import ml_dtypes

    f8 = ml_dtypes.float8_e4m3
    x = np.ascontiguousarray(np.asarray(x, dtype=np.float32))
    W1 = np.asarray(W1, dtype=np.float32)
    W2 = np.asarray(W2, dtype=np.float32)
    W0 = np.asarray(W0, dtype=np.float32)

    w1aT = np.ascontiguousarray(W1[:, :F].T).astype(np.float16)        # [F, H]
    w1bT = np.ascontiguousarray(W1[:, F:].T * WSCALE).astype(f8)       # [F, H]
    w2T = np.ascontiguousarray(W2.T).astype(np.float16)                # [H, F]
    w0T = np.ascontiguousarray(W0.T).astype(np.float16)                # [F, F]

    # bmask[r, q*1024 + b*256 + n] = WSCALE iff r == q*4+b and n != 255
    bmask = np.zeros((BL, QUADS, 4, NP), dtype=np.float16)
    for qq in range(QUADS):
        for bb in range(4):
            bmask[qq * 4 + bb, qq, bb, :NI] = WSCALE
    bmask = bmask.reshape(BL, QUADS * 1024)

    in_maps = []
    for i in range(N_CORES):
        xc = x[i * BL : (i + 1) * BL]               # [BL, N, F]
        # packed [128, NF*BL]: row p, block f holds x0T[f*128+p, :]
        x0Tc = np.ascontiguousarray(
            xc[:, 0, :].T.reshape(NF, 128, BL).transpose(1, 0, 2).reshape(128, NF * BL)
        ).astype(np.float16)
        pad = np.zeros((BL, NP, F), dtype=np.float32)
        pad[:, :NI, :] = xc[:, 1:, :]
        xiTc = np.ascontiguousarray(pad.reshape(BL * NP, F).T).astype(f8)
        in_maps.append(
            {
                "xiT": xiTc,
                "x0T": x0Tc,
                "w1bT": w1bT,
                "w1aT": w1aT,
                "w2T": w2T,
                "w0T": w0T,
                "bmaskT": bmask,
            }
        )
    return in_maps


def _gather(results):
    out = np.empty((B, F), dtype=np.float32)
    for i in range(N_CORES):
        out[i * BL : (i + 1) * BL] = results[i]["res"]
    return out


def kernel(x, W1, W2, W0):
    nc = _get_nc()
    in_maps = _make_in_maps(x, W1, W2, W0)
    res = run_bass_kernel_spmd(nc, in_maps, list(range(N_CORES)))
    return _gather(res.results)


def kernel_profiled(x, W1, W2, W0, **trace_kwargs):
    """Like kernel() but with NTFF profiling; returns (out, exec_time_ns)."""
    nc = _get_nc()
    in_maps = _make_in_maps(x, W1, W2, W0)
    res = run_bass_kernel_spmd(
        nc, in_maps, list(range(N_CORES)), trace=True, **trace_kwargs
    )
    return _gather(res.results), res.exec_time_ns


# revision 18
# speedup vs baseline: 1.1093x; 1.1089x over previous
"""Trainium2 Bass kernel for nn_Attention_39934605918652.

res[b] = W0 @ x0[b] + sum_{n=1..N-1} W2 @ tanh(W1a @ x0[b] + W1b @ x[b,n])

Key algebraic optimization: W2 does not depend on n, so
    sum_n W2 @ tanh(...) = W2 @ (sum_n tanh(...))
which removes the second big matmul (only a [B,H]x[H,F] remains).

Sharding: data-parallel over batch B=128 across 8 cores (16 batches/core),
weights replicated. No collectives.

The dominant [F=512]-contraction matmul runs in fp8 e4m3 DoubleRow mode
(256 contraction rows per instruction; measured 215ns per 512-col matmul
warm = 2x bf16). W1b is host-scaled by 32 so its N(0, 1/1024) entries use
the e4m3 range; the tanh compensates via the ACT scale=1/32 immediate.

Engine budget (measured): the PE+ACT pair carries a conserved ~2.5us per
quad-tile (bias via ACT costs 4 small calls = 1.59us ACT; bias via a K=16
one-hot PE matmul costs 0.63us PE + 1.04us big-call ACT), so the mix knob
KB_NS1 balances them. ACT is not subject to the PE's HAM clock throttle,
so the default mix makes ACT the steady-state pacer. The segmented
free-dim reduce runs on DVE, with a GpSimd halving pre-add (SBUF-only;
GPSIMD cannot touch PSUM, cannot reduce, runs elementwise at 0.42 eff)
offloading KB_NGH of the 32 tiles.

All DRAM tensors are host-packed so every SBUF tile loads with ONE
contiguous dma_start of >=2KB-per-partition rows (small descriptors
measured ~40% DMA throughput loss):
  xiQ   [8*128, 2048] fp8   row (fp*4+q)*128+p = xi[f=fp*256+i*128+p,
                            q*1024+c] pairs (i,c)-major; pad col n=255
  w1bQ  [2*128, 2048] fp8   (= 32*W1b.T, DoubleRow pair layout per fp)
  x0T   [128, 4*16]   fp16  host-packed f-chunks side by side
  x0Q8  [128, 4*16]   fp8   same, for the fp8 W0-term matmuls
  w1aT  [512, 1024]   fp16  (= W1a.T)
  w2Q   [4*128, 1024] fp16  h-tile pairs side by side (= W2.T regrouped)
  w0Q   [128, 2048]   fp8   f-chunks side by side (= W0.T regrouped)
  bmask [16, 4*1024]  fp16  one-hot bias mask per quad: bmask[r, q*1024+
                            b*256+n] = 32 iff r == q*4+b and n != 255
Output res [BL=16, F=512] per core (batch-major); host concatenates.
"""

import os
import numpy as np
from contextlib import ExitStack

import concourse.bass as bass
import concourse.tile as tile
from concourse import bacc, mybir
from concourse.bass_utils import run_bass_kernel_spmd

N_CORES = 8
B, N, F, H = 128, 256, 512, 1024
BL = B // N_CORES          # 16 batches per core
NI = N - 1                 # 255 real columns per batch
NP = 256                   # padded columns per batch
NF = F // 128              # 4 f-chunks
FP = 2                     # 2 f-pair chunks (256 rows each, DoubleRow)
NH = H // 128              # 8 h-tiles
QUADS = BL // 4            # 4 batch-quads; per quad psum tile [128, 4*256]
QW = 4 * NP                # 1024 columns per quad
WSCALE = 32.0              # host-side W1b/bias scale (ACT scale=1/32)

F32 = mybir.dt.float32
BF16 = mybir.dt.bfloat16
F16 = mybir.dt.float16
F8 = mybir.dt.float8e4
DR = mybir.MatmulPerfMode.DoubleRow

# Knobs (sweepable on hw):
#  KB_NS1A: tiles using S1a = 4 per-batch fused-bias ACT calls with the
#          ACT accumulator producing S directly (no reduce; +187ns engine
#          time per call for the accumulator read).
#  KB_NS1: tiles using S1 = 4 per-batch fused-bias ACT calls + reduce.
#          Remaining tiles use S4 = PE one-hot bias matmul + one big
#          1024-col plain tanh + reduce. The last TAIL_S4 tiles are
#          forced S4 (single tanh call drains the pipeline fastest).
#  KB_NGH: reduces prefaced by a GpSimd halving add, then a half-width
#          DVE reduce. Rest: plain DVE reduce_sum.
#  KB_NTTR: DVE tensor_tensor_reduce variant — measured on hw: the
#          instruction wedges the device (NRT unrecoverable). Leave 0.
#  KB_WARM: dummy [128,128] f32 matmuls (426ns each: fp32 = 2 passes)
#          during the DMA lead-in to keep the PE clock governor warm.
#  KB_PPB: main PSUM pool bufs ([128,1024] f32 slots, 2 banks each).
NS1A = int(os.environ.get("KB_NS1A", "0"))
NS1 = int(os.environ.get("KB_NS1", "16"))
NTTR = int(os.environ.get("KB_NTTR", "0"))
NGH = int(os.environ.get("KB_NGH", "22"))
WARM_N = int(os.environ.get("KB_WARM", "9"))
PPB = int(os.environ.get("KB_PPB", "4"))
TAIL_S4 = int(os.environ.get("KB_TAIL", "5"))


def _build_kernel():
    nc = bacc.Bacc(
        "TRN2", target_bir_lowering=False, debug=False, num_devices=N_CORES
    )

    xiQ = nc.dram_tensor("xiQ", [FP * QUADS * 128, 2048], F8, kind="ExternalInput").ap()
    w1bQ = nc.dram_tensor("w1bQ", [FP * 128, 2048], F8, kind="ExternalInput").ap()
    x0T = nc.dram_tensor("x0T", [128, NF * BL], F16, kind="ExternalInput").ap()
    x0Q8 = nc.dram_tensor("x0Q8", [128, NF * BL], F8, kind="ExternalInput").ap()
    w1aT = nc.dram_tensor("w1aT", [F, H], F16, kind="ExternalInput").ap()
    w2Q = nc.dram_tensor("w2Q", [NF * 128, 1024], F16, kind="ExternalInput").ap()
    w0Q = nc.dram_tensor("w0Q", [128, 2048], F8, kind="ExternalInput").ap()
    bmaskT = nc.dram_tensor(
        "bmaskT", [BL, QUADS * 1024], F16, kind="ExternalInput"
    ).ap()
    res = nc.dram_tensor("res", [BL, F], F32, kind="ExternalOutput").ap()

    with tile.TileContext(nc) as tc:
        with ExitStack() as ctx:
            _kernel_body(
                ctx, tc, xiQ, w1bQ, x0T, x0Q8, w1aT, w2Q, w0Q, bmaskT, res
            )

    nc.compile()
    return nc


def _kernel_body(ctx, tc, xiQ, w1bQ, x0T, x0Q8, w1aT, w2Q, w0Q, bmaskT, res):
    nc = tc.nc
    Tanh = mybir.ActivationFunctionType.Tanh

    wpool = ctx.enter_context(tc.tile_pool(name="weights", bufs=1))

    def load_rows(name, dram, r0, shape, dt):
        t = wpool.tile(shape, dt, tag=name, name=name)
        flat = t[:] if len(shape) == 2 else t[:].rearrange("p a b -> p (a b)")
        nc.sync.dma_start(flat, dram[r0 : r0 + shape[0], :])
        return t

    # ---- DMA issue order = first-need order ----
    x0_all = load_rows("x0", x0T, 0, [128, NF * BL], F16)
    x0_sb = [x0_all[:, f * BL : (f + 1) * BL] for f in range(NF)]
    x08_all = load_rows("x08", x0Q8, 0, [128, NF * BL], F8)
    x08_sb = [x08_all[:, f * BL : (f + 1) * BL] for f in range(NF)]
    w1b_sb = [
        load_rows(f"w1b_{fp}", w1bQ, fp * 128, [128, 2, 1024], F8)
        for fp in range(FP)
    ]
    xi_sb = [[None] * QUADS for _ in range(FP)]
    for q in range(2):  # wave-0 quads first
        for fp in range(FP):
            xi_sb[fp][q] = load_rows(
                f"xi_{fp}_{q}", xiQ, (fp * QUADS + q) * 128, [128, 2, 1024], F8
            )
    w1a_sb = [
        load_rows(f"w1a_{c}", w1aT, c * 128, [128, H], F16) for c in range(NF)
    ]
    bmask_sb = load_rows("bmask", bmaskT, 0, [BL, QUADS * 1024], F16)
    w0_sb = load_rows("w0", w0Q, 0, [128, 2048], F8)
    for q in (2, 3):  # wave-1 quads stream during wave 0
        for fp in range(FP):
            xi_sb[fp][q] = load_rows(
                f"xi_{fp}_{q}", xiQ, (fp * QUADS + q) * 128, [128, 2, 1024], F8
            )
    w2_sb = [
        load_rows(f"w2_{j}", w2Q, j * 128, [128, 1024], F16) for j in range(NF)
    ]

    def w2_slice(h):
        return w2_sb[h // 2][:, (h % 2) * 512 : (h % 2 + 1) * 512]

    h0_sb = [
        wpool.tile([128, BL], F32, tag=f"h0_{h}", name=f"h0_{h}")
        for h in range(NH)
    ]
    h0T_sb = wpool.tile([BL, H], F16, tag="h0T", name="h0T")
    S_sb = [
        wpool.tile([128, BL], F16, tag=f"S_{h}", name=f"S_{h}")
        for h in range(NH)
    ]

    # One PSUM pool; every tile shares the tag so slots recycle.
    # Slot = [128, 4*NP] f32 = 2 banks; PPB slots = the full 8 banks.
    # Warm-up, phase 1, and the epilogue matmul groups borrow slots
    # transiently; the epilogue accumulates in SBUF via DVE.
    ppool = ctx.enter_context(tc.tile_pool(name="ps", bufs=PPB, space="PSUM"))
    itpool = ctx.enter_context(tc.tile_pool(name="it", bufs=6))

    # ---- Phase 0: PE warm-up during the DMA lead-in ----
    if WARM_N:
        wz = wpool.tile([128, 128], F32, tag="warmz", name="warmz")
        nc.vector.memset(wz[:], 0.0)
        pw = ppool.tile([128, 128], F32, tag="ps", name="pwarm")
        for _ in range(WARM_N):
            nc.tensor.matmul(pw[:], wz[:], wz[:], start=True, stop=True)

    # ---- Phase 0b: preload the tanh ACT table during the DMA lead-in
    tiny = wpool.tile([128, 1], F32, tag="tiny", name="tiny")
    nc.vector.memset(tiny[:], 0.0)
    nc.scalar.activation(tiny[:], tiny[:], Tanh)

    # ---- Phase 1 (issued after the LEAD wave-0 tiles; see below):
    def phase1a():
        # h0[h*128+p, b] = sum_f W1a[h, f] x0[b, f]; [128, BL] per h-tile.
        for h in range(NH):
            ph = ppool.tile([128, BL], F32, tag="ps", name=f"ph0_{h}")
            for f in range(NF):
                nc.tensor.matmul(
                    ph[:],
                    w1a_sb[f][:, h * 128 : (h + 1) * 128],
                    x0_sb[f],
                    start=(f == 0),
                    stop=(f == NF - 1),
                )
            nc.vector.tensor_copy(h0_sb[h][:], ph[:])

    def phase1b():
        # h0T[b, h] flipped variant for the S4 bias matmuls' stationary.
        ph = ppool.tile([BL, H], F32, tag="ps", name="ph0T")
        for hb in range(2):
            for f in range(NF):
                nc.tensor.matmul(
                    ph[:, hb * 512 : (hb + 1) * 512],
                    x0_sb[f],
                    w1a_sb[f][:, hb * 512 : (hb + 1) * 512],
                    start=(f == 0),
                    stop=(f == NF - 1),
                )
        with nc.allow_low_precision(reason="h0T feeds fp16 bias matmul"):
            nc.vector.tensor_copy(h0T_sb[:], ph[:])

    # ---- Phase 3: epilogue res = W0 x0 + W2 S, accumulated in SBUF.
    # Each 4-matmul group borrows a psum slot transiently and DVE folds
    # it into rt_acc, so no slot is held across phase 2.
    rt_acc = wpool.tile([BL, F], F32, tag="rt", name="rt_acc")

    def epilogue_w0():
        pw = ppool.tile([BL, F], F32, tag="ps", name="po_w0")
        for f in range(NF):
            nc.tensor.matmul(
                pw[:],
                x08_sb[f],
                w0_sb[:, f * 512 : (f + 1) * 512],
                start=(f == 0),
                stop=(f == NF - 1),
            )
        nc.vector.tensor_copy(rt_acc[:], pw[:])

    def epilogue_s_group(hs, name):
        pg = ppool.tile([BL, F], F32, tag="ps", name=name)
        for i, h in enumerate(hs):
            nc.tensor.matmul(
                pg[:], S_sb[h][:], w2_slice(h),
                start=(i == 0), stop=(i == len(hs) - 1),
            )
        nc.vector.tensor_add(rt_acc[:], rt_acc[:], pg[:])

    # ---- Phase 2: hi matmul (fp8 DoubleRow) + bias + tanh + reduce ----
    def consume(h, q, pb, cls, red):
        it = itpool.tile([128, 4 * NP], BF16, tag="it", name=f"it_{h}_{q}")
        s1ish = cls in ("s1", "s1a")
        nb = NI if s1ish else NP
        with nc.allow_low_precision(
            reason="S accumulated in 16-bit to feed the 16-bit output matmul"
        ):
            if s1ish:
                # per-batch ACT, bias via the ACT bias port, skip pad col;
                # s1a also reads the ACT accumulator = the row sum = S col.
                for bl in range(4):
                    b = q * 4 + bl
                    acc = S_sb[h][:, b : b + 1] if cls == "s1a" else None
                    nc.scalar.activation(
                        it[:, bl * NP : bl * NP + NI],
                        pb[:, bl * NP : bl * NP + NI],
                        Tanh,
                        bias=h0_sb[h][:, b : b + 1],
                        scale=1.0 / WSCALE,
                        accum_out=acc,
                    )
                if cls == "s1a":
                    return
            else:
                # S4: bias already in PSUM (one-hot matmul, pad col exact 0
                # since bmask zeroes it and tanh(0)=0): one big tanh call.
                nc.scalar.activation(it[:], pb[:], Tanh, scale=1.0 / WSCALE)
            scol = S_sb[h][:, q * 4 : (q + 1) * 4]
            view = it[:].rearrange("p (b n) -> p b n", b=4)
            if red == "gph":
                # GpSimd halving add (SBUF-only), then half-width DVE reduce.
                hb = nb // 2
                nc.gpsimd.tensor_add(
                    view[:, :, :hb],
                    view[:, :, :hb],
                    view[:, :, nb - hb : nb],
                )
                nc.vector.reduce_sum(
                    scol, view[:, :, : nb - hb], axis=mybir.AxisListType.X
                )
            else:
                nc.vector.reduce_sum(
                    scol, view[:, :, :nb], axis=mybir.AxisListType.X
                )

    def mm_main(pb, h, q, s1ish):
        # 2 DoubleRow matmuls per 512-col block: fpair 0 starts, fpair 1
        # accumulates; S4 groups stay open for the bias matmul.
        for bk in range(2):
            out = pb[:, bk * 512 : (bk + 1) * 512]
            for fp in range(FP):
                nc.tensor.matmul(
                    out,
                    w1b_sb[fp][:, :, h * 128 : (h + 1) * 128],
                    xi_sb[fp][q][:, :, bk * 512 : (bk + 1) * 512],
                    start=(fp == 0),
                    stop=(fp == FP - 1) and s1ish,
                    perf_mode=DR,
                )

    def mm_bias(pb, h, q):
        for bk in range(2):
            nc.tensor.matmul(
                pb[:, bk * 512 : (bk + 1) * 512],
                h0T_sb[:, h * 128 : (h + 1) * 128],
                bmask_sb[:, q * 1024 + bk * 512 : q * 1024 + (bk + 1) * 512],
                start=False,
                stop=True,
            )

    # PE program order: LEAD wave-0 tiles first (gated only on w1b +
    # xi-q0/q1, ~1MB of DMA), then phase 1 (w1a streams meanwhile), then
    # the rest. Wave 1 walks h DESCENDING so S[7..4] complete early and
    # their epilogue group issues mid-stream; only {3..0} trails the
    # final consume.
    sched = []
    for wave in range(QUADS // 2):
        hs = range(NH) if wave == 0 else range(NH - 1, -1, -1)
        for h in hs:
            for q in (2 * wave, 2 * wave + 1):
                sched.append((wave, h, q))

    LEAD = 3  # <= PPB-1: lead tiles hold slots; phase 1 borrows the last

    # Deficit-spread class labels; first LEAD tiles non-S4 (their matmuls
    # precede h0T in the PE stream), last TAIL_S4 tiles forced S4.
    counts = {"s1a": NS1A, "s1": NS1}
    if counts["s1a"] + counts["s1"] < LEAD:
        counts["s1"] = LEAD - counts["s1a"]
    counts["s4"] = 32 - counts["s1a"] - counts["s1"]
    labels = []
    used = {k: 0 for k in counts}
    for pos in range(32):
        opts = [k for k in counts if used[k] < counts[k]]
        if pos < LEAD:
            opts = [k for k in opts if k != "s4"] or ["s1a"]
        elif pos >= 32 - TAIL_S4 and used["s4"] < counts["s4"]:
            opts = ["s4"]
        pick = max(opts, key=lambda k: counts[k] * (pos + 1) / 32 - used[k])
        used[pick] = used.get(pick, 0) + 1
        labels.append(pick)
    nred = sum(1 for l in labels if l != "s1a")
    rcounts = {"ttr": min(NTTR, nred), "gph": min(NGH, max(nred - NTTR, 0))}
    rcounts["plain"] = nred - rcounts["ttr"] - rcounts["gph"]
    rlabels = []
    rused = {k: 0 for k in rcounts}
    for pos in range(nred):
        opts = [k for k in rcounts if rused[k] < rcounts[k]]
        pick = max(opts, key=lambda k: rcounts[k] * (pos + 1) / nred - rused[k])
        rused[pick] += 1
        rlabels.append(pick)
    riter = iter(rlabels)
    plan = [(l, next(riter) if l != "s1a" else None) for l in labels]

    deferred = []
    for pos, (wave, h, q) in enumerate(sched):
        if pos == LEAD:
            phase1a()
            phase1b()
            for dh, dq, dpb, dcls, dred in deferred:
                if dcls == "s4":
                    mm_bias(dpb, dh, dq)
                consume(dh, dq, dpb, dcls, dred)
        cls, red = plan[pos]
        pb = ppool.tile([128, 4 * NP], F32, tag="ps", name=f"pb_{h}_{q}")
        mm_main(pb, h, q, cls in ("s1", "s1a"))
        if pos < LEAD:
            deferred.append((h, q, pb, cls, red))
            continue
        if cls == "s4":
            mm_bias(pb, h, q)
        consume(h, q, pb, cls, red)
        if pos == 11:
            epilogue_w0()
        if pos == 25:
            epilogue_s_group([7, 6, 5, 4], "po_sA")

    epilogue_s_group([3, 2, 1, 0], "po_sB")
    nc.sync.dma_start(res[:], rt_acc[:])


_NC_CACHE = {}


def _get_nc():
    key = ("v6", NS1A, NS1, NTTR, NGH, WARM_N, PPB, TAIL_S4)
    if key not in _NC_CACHE:
        _NC_CACHE[key] = _build_kernel()
    return _NC_CACHE[key]


def _make_in_maps(x, W1, W2, W0):
    import ml_dtypes

    f8 = ml_dtypes.float8_e4m3
    x = np.ascontiguousarray(np.asarray(x, dtype=np.float32))
    W1 = np.asarray(W1, dtype=np.float32)
    W2 = np.asarray(W2, dtype=np.float32)
    W0 = np.asarray(W0, dtype=np.float32)

    w1aT = np.ascontiguousarray(W1[:, :F].T).astype(np.float16)       # [F, H]
    w1bT = (W1[:, F:].T * WSCALE).astype(f8)                          # [F, H]
    # DoubleRow pair layout: row fp*128+p = [i0 h0..1023, i1 h0..1023]
    w1bQ = np.ascontiguousarray(
        w1bT.reshape(FP, 2, 128, H).transpose(0, 2, 1, 3).reshape(FP * 128, 2 * H)
    )
    w2T = np.ascontiguousarray(W2.T).astype(np.float16)               # [H, F]
    w2Q = np.ascontiguousarray(
        w2T.reshape(NF, 2, 128, F).transpose(0, 2, 1, 3).reshape(NF * 128, 2 * F)
    )
    w0T = np.ascontiguousarray(W0.T).astype(f8)                       # [F, F]
    w0Q = np.ascontiguousarray(
        w0T.reshape(NF, 128, F).transpose(1, 0, 2).reshape(128, NF * F)
    )

    # bmask[r, q*1024 + b*256 + n] = WSCALE iff r == q*4+b and n != 255
    bmask = np.zeros((BL, QUADS, 4, NP), dtype=np.float16)
    for qq in range(QUADS):
        for bb in range(4):
            bmask[qq * 4 + bb, qq, bb, :NI] = WSCALE
    bmask = bmask.reshape(BL, QUADS * 1024)

    in_maps = []
    for i in range(N_CORES):
        xc = x[i * BL : (i + 1) * BL]               # [BL, N, F]
        # packed [128, NF*BL]: row p, block f holds x0T[f*128+p, :]
        x0p = np.ascontiguousarray(
            xc[:, 0, :].T.reshape(NF, 128, BL).transpose(1, 0, 2).reshape(128, NF * BL)
        )
        pad = np.zeros((BL, NP, F), dtype=np.float32)
        pad[:, :NI, :] = xc[:, 1:, :]
        xiT = pad.reshape(BL * NP, F).T.astype(f8)  # [F, BL*NP]
        # row (fp*4+q)*128+p = [i0 c0..1023, i1 c0..1023] of quad q
        xiQ = np.ascontiguousarray(
            xiT.reshape(FP, 2, 128, QUADS, QW)
            .transpose(0, 3, 2, 1, 4)
            .reshape(FP * QUADS * 128, 2 * QW)
        )
        in_maps.append(
            {
                "xiQ": xiQ,
                "x0T": x0p.astype(np.float16),
                "x0Q8": x0p.astype(f8),
                "w1bQ": w1bQ,
                "w1aT": w1aT,
                "w2Q": w2Q,
                "w0Q": w0Q,
                "bmaskT": bmask,
            }
        )
    return in_maps


def _gather(results):
    out = np.empty((B, F), dtype=np.float32)
    for i in range(N_CORES):
        out[i * BL : (i + 1) * BL] = results[i]["res"]
    return out


def kernel(x, W1, W2, W0):
    nc = _get_nc()
    in_maps = _make_in_maps(x, W1, W2, W0)
    res = run_bass_kernel_spmd(nc, in_maps, list(range(N_CORES)))
    return _gather(res.results)


def kernel_profiled(x, W1, W2, W0, **trace_kwargs):
    """Like kernel() but with NTFF profiling; returns (out, exec_time_ns)."""
    nc = _get_nc()
    in_maps = _make_in_maps(x, W1, W2, W0)
    res = run_bass_kernel_spmd(
        nc, in_maps, list(range(N_CORES)), trace=True, **trace_kwargs
    )
    return _gather(res.results), res.exec_time_ns
